# revision 1
# baseline (speedup 1.0000x reference)
"""Trainium2 Bass kernel for the Deep OSTL model.

Model (per reference):
    z = x @ proj_W.T + proj_b
    for l in 0..3:
        zx = z @ Wx[l].T + bz[l]
        h = 0; 8x: h = tanh(h @ Wz[l].T + zx)
        z = h
    out = z @ head_W.T + head_b

Structure exploited: Wz[l] == 0.5*I (checked at runtime; numpy fallback
otherwise), so the recurrence is elementwise: h = tanh(0.5*h + zx).

Device layout: feature-major (transposed). Each of the 8 cores gets a
4096-row batch shard as x.T [512, 4096]. All matmuls contract over the
partition dim in 128-chunks, fp32r (full PE rate at free dim 512).
Recurrence runs in-place on SBUF: DVE scalar_tensor_tensor (u = 0.5*h + zx)
+ ACT tanh, per 128x4096 hidden-chunk.
"""

import sys
from contextlib import ExitStack

import numpy as np

sys.path.insert(0, "/opt/trn_rl_repo")

# ---- problem constants (hardcoded per contract) ----
B = 32768           # total batch
D = 512             # in/hidden/out dim
L = 4               # layers
T = 8               # recurrence steps
NCORES = 8
BC = B // NCORES    # per-core batch (4096)
KC = D // 128       # hidden chunks (4)
HALF = BC // 2      # 2048, psum-tile free width
NMM = HALF // 512   # 512-wide matmul slices per psum tile (4)
PE_STEPS = {2, 4, 6}    # recurrence steps offloaded to the tensor engine
PE_STEPS_L0 = {4, 6}    # layer 0: keep early steps off the PE while it ramps

_STATE = {}


def _build(reps: int = 1):
    import concourse.bacc as bacc
    import concourse.mybir as mybir
    from concourse import tile

    fp32 = mybir.dt.float32
    fp32r = mybir.dt.float32r
    Alu = mybir.AluOpType
    Act = mybir.ActivationFunctionType

    nc = bacc.Bacc("TRN2")

    xt = nc.dram_tensor("xt", [D, BC], fp32r, kind="ExternalInput").ap()
    pwt = nc.dram_tensor("pwt", [D, D], fp32r, kind="ExternalInput").ap()
    pb = nc.dram_tensor("pb", [D, 1], fp32, kind="ExternalInput").ap()
    wxt = nc.dram_tensor("wxt", [L, D, D], fp32r, kind="ExternalInput").ap()
    bz = nc.dram_tensor("bz", [L, D, 1], fp32, kind="ExternalInput").ap()
    hwt = nc.dram_tensor("hwt", [D, D], fp32r, kind="ExternalInput").ap()
    hb = nc.dram_tensor("hb", [D, 1], fp32, kind="ExternalInput").ap()
    eye = nc.dram_tensor("eye", [128, 256], fp32r, kind="ExternalInput").ap()
    outt = nc.dram_tensor("outt", [D, BC], fp32, kind="ExternalOutput").ap()

    with tile.TileContext(nc) as tc, ExitStack() as ctx:
        wpool = ctx.enter_context(tc.tile_pool(name="weights", bufs=1))
        wxpool = ctx.enter_context(tc.tile_pool(name="wxp", bufs=2))
        state = ctx.enter_context(tc.tile_pool(name="state", bufs=1))
        iop = ctx.enter_context(tc.tile_pool(name="io", bufs=4))
        psp = ctx.enter_context(tc.tile_pool(name="ps", bufs=2, space="PSUM"))

        # state tiles first so their pool slots are stable
        HT = state.tile([128, KC * BC], fp32r, tag="ht", name="HT")
        ZXT = state.tile([128, KC * BC], fp32r, tag="zxt", name="ZXT")
        Hs = [HT[:, c * BC:(c + 1) * BC] for c in range(KC)]
        ZXs = [ZXT[:, c * BC:(c + 1) * BC] for c in range(KC)]

        PW = wpool.tile([128, KC * D], fp32r, tag="pw")
        HW = wpool.tile([128, KC * D], fp32r, tag="hw")
        PB = wpool.tile([128, KC], fp32, tag="pb")
        HB = wpool.tile([128, KC], fp32, tag="hb")
        BZ = wpool.tile([128, L * KC], fp32, tag="bz")
        EYE = wpool.tile([128, 256], fp32r, tag="eye")

        warm = wpool.tile([128, 1], fp32, tag="warm")
        nc.gpsimd.memset(warm[:], 0.0)
        nc.scalar.activation(warm[:], warm[:], Act.Tanh)

        for rep in range(reps):
            # ---- input x.T tiles: DMA'd in 512-wide pieces for early MM start
            xts = {}
            for half in range(2):
                for k in range(KC):
                    xts[half, k] = iop.tile([128, HALF], fp32r, tag="io",
                                            name=f"xt_{rep}_{half}_{k}")
                for i in range(NMM):
                    for k in range(KC):
                        eng = (nc.sync, nc.scalar, nc.gpsimd)[(i * KC + k) % 3]
                        eng.dma_start(
                            xts[half, k][:, i * 512:(i + 1) * 512],
                            xt[k * 128:(k + 1) * 128,
                               half * HALF + i * 512:half * HALF + (i + 1) * 512])
                if rep == 0 and half == 0:
                    # weights for proj + layer 1 arrive after the first x half
                    for k in range(KC):
                        nc.gpsimd.dma_start(PW[:, k * D:(k + 1) * D],
                                            pwt[k * 128:(k + 1) * 128, :])
                    nc.gpsimd.dma_start(
                        PB[:], pb.rearrange("(o p) x -> p (o x)", p=128))
                    nc.gpsimd.dma_start(
                        BZ[:], bz.rearrange("l (o p) x -> p (l o x)", p=128))
                    nc.gpsimd.dma_start(EYE[:], eye[:])

            # ---- proj + layers, half-pipelined emission ----
            def mm_group(wt, woff, rhs_of, o, half, pname):
                P = psp.tile([128, HALF], fp32, tag="ps", name=pname)
                for i in range(NMM):
                    for k in range(KC):
                        nc.tensor.matmul(
                            P[:, i * 512:(i + 1) * 512],
                            wt[:, woff + k * D + o * 128: woff + k * D + (o + 1) * 128],
                            rhs_of(k, half * HALF + i * 512),
                            start=(k == 0), stop=(k == KC - 1),
                        )
                return P

            x_rhs = lambda k, fo: xts[fo // HALF, k][:, fo % HALF:fo % HALF + 512]
            h_rhs = lambda k, fo: Hs[k][:, fo:fo + 512]

            WX0 = wxpool.tile([128, KC * D], fp32r, tag="wx", name=f"WX{rep}_0")
            for half in range(2):
                for q in range(2):
                    base = half * HALF + q * 1024
                    for o in range(KC):
                        P = psp.tile([128, 1024], fp32, tag="ps",
                                     name=f"pp_{rep}_{half}_{q}_{o}")
                        for i in range(2):
                            for k in range(KC):
                                nc.tensor.matmul(
                                    P[:, i * 512:(i + 1) * 512],
                                    PW[:, k * D + o * 128: k * D + (o + 1) * 128],
                                    xts[half, k][:, q * 1024 + i * 512:
                                                 q * 1024 + (i + 1) * 512],
                                    start=(k == 0), stop=(k == KC - 1),
                                )
                        nc.vector.tensor_scalar_add(
                            Hs[o][:, base:base + 1024], P[:], PB[:, o:o + 1])
                    if half == 0 and q == 0:
                        for k in range(KC):
                            nc.sync.dma_start(WX0[:, k * D:(k + 1) * D],
                                              wxt[0, k * 128:(k + 1) * 128, :])
                        if rep == 0:
                            nc.sync.dma_start(
                                HB[:], hb.rearrange("(o p) x -> p (o x)", p=128))
                    # layer-1 zx matmuls for this quarter, right behind proj
                    for o in range(KC):
                        P = psp.tile([128, 1024], fp32, tag="ps",
                                     name=f"zp0_{rep}_{half}_{q}_{o}")
                        for i in range(2):
                            for k in range(KC):
                                nc.tensor.matmul(
                                    P[:, i * 512:(i + 1) * 512],
                                    WX0[:, k * D + o * 128: k * D + (o + 1) * 128],
                                    Hs[k][:, base + i * 512:base + (i + 1) * 512],
                                    start=(k == 0), stop=(k == KC - 1),
                                )
                        nc.vector.tensor_scalar_add(
                            ZXs[o][:, base:base + 1024], P[:], BZ[:, o:o + 1])
                    for c in range(KC):
                        nc.scalar.activation(Hs[c][:, base:base + 1024],
                                             ZXs[c][:, base:base + 1024], Act.Tanh)

            # ---- layers; each half's next-phase matmuls interleave with
            # the other half's last recurrence step ----
            def emit_input_phase(l, half):
                """zx matmuls + drains + step-1 tanh for (layer l, half)."""
                WXl = wx_tiles[l]
                for o in range(KC):
                    P = mm_group(WXl, 0, h_rhs, o, half, f"zp_{rep}_{l}_{half}_{o}")
                    nc.vector.tensor_scalar_add(
                        ZXs[o][:, half * HALF:(half + 1) * HALF], P[:],
                        BZ[:, l * KC + o:l * KC + o + 1])
                for c in range(KC):
                    sl = slice(half * HALF, (half + 1) * HALF)
                    nc.scalar.activation(Hs[c][:, sl], ZXs[c][:, sl], Act.Tanh)

            def emit_head_phase(half, q):
                # quarter-grain (1024) head so the post-recurrence tail is short
                base = half * HALF + q * 1024
                for o in range(KC):
                    P = psp.tile([128, 1024], fp32, tag="ps",
                                 name=f"hp_{rep}_{half}_{q}_{o}")
                    for i in range(2):
                        for k in range(KC):
                            nc.tensor.matmul(
                                P[:, i * 512:(i + 1) * 512],
                                HW[:, k * D + o * 128: k * D + (o + 1) * 128],
                                Hs[k][:, base + i * 512:base + (i + 1) * 512],
                                start=(k == 0), stop=(k == KC - 1),
                            )
                    O = iop.tile([128, 1024], fp32, tag="ioq", bufs=3,
                                 name=f"O_{rep}_{half}_{q}_{o}")
                    nc.vector.tensor_scalar_add(O[:], P[:], HB[:, o:o + 1])
                    (nc.sync if o % 2 == 0 else nc.gpsimd).dma_start(
                        outt[o * 128:(o + 1) * 128, base:base + 1024], O[:])

            wx_tiles = {0: WX0}
            for l in range(L):
                if l + 1 < L:
                    WXn = wxpool.tile([128, KC * D], fp32r, tag="wx",
                                      name=f"WX{rep}_{l + 1}")
                    for k in range(KC):
                        nc.sync.dma_start(WXn[:, k * D:(k + 1) * D],
                                          wxt[l + 1, k * 128:(k + 1) * 128, :])
                    wx_tiles[l + 1] = WXn
                    if rep == 0 and l == L - 2:
                        for k in range(KC):
                            nc.sync.dma_start(HW[:, k * D:(k + 1) * D],
                                              hwt[k * 128:(k + 1) * 128, :])
                pe_steps = PE_STEPS if l > 0 else PE_STEPS_L0
                for t in range(1, T - 1):
                    if t in pe_steps:
                        for half in range(2):
                            for c in range(KC):
                                P = psp.tile([128, HALF], fp32, tag="ps",
                                             name=f"fma_{rep}_{l}_{t}_{c}_{half}")
                                for i in range(NMM):
                                    fo = half * HALF + i * 512
                                    sl = P[:, i * 512:(i + 1) * 512]
                                    nc.tensor.matmul(sl, EYE[:, 0:128],
                                                     ZXs[c][:, fo:fo + 512],
                                                     start=True, stop=False)
                                    nc.tensor.matmul(sl, EYE[:, 128:256],
                                                     Hs[c][:, fo:fo + 512],
                                                     start=False, stop=True)
                                nc.scalar.activation(
                                    Hs[c][:, half * HALF:(half + 1) * HALF],
                                    P[:], Act.Tanh)
                    else:
                        for half in range(2):
                            for c in range(KC):
                                sl = slice(half * HALF, (half + 1) * HALF)
                                nc.vector.scalar_tensor_tensor(
                                    Hs[c][:, sl], Hs[c][:, sl], 0.5,
                                    ZXs[c][:, sl], Alu.mult, Alu.add)
                                nc.scalar.activation(Hs[c][:, sl], Hs[c][:, sl],
                                                     Act.Tanh)
                # last step (t = T-1) per half, with the next phase for that
                # half emitted immediately behind it
                if l + 1 < L:
                    for half in range(2):
                        for c in range(KC):
                            sl = slice(half * HALF, (half + 1) * HALF)
                            nc.vector.scalar_tensor_tensor(
                                Hs[c][:, sl], Hs[c][:, sl], 0.5,
                                ZXs[c][:, sl], Alu.mult, Alu.add)
                            nc.scalar.activation(Hs[c][:, sl], Hs[c][:, sl],
                                                 Act.Tanh)
                        emit_input_phase(l + 1, half)
                else:
                    # final layer: quarter-grain last step + head
                    for half in range(2):
                        for q in range(2):
                            for c in range(KC):
                                sl = slice(half * HALF + q * 1024,
                                           half * HALF + (q + 1) * 1024)
                                nc.vector.scalar_tensor_tensor(
                                    Hs[c][:, sl], Hs[c][:, sl], 0.5,
                                    ZXs[c][:, sl], Alu.mult, Alu.add)
                                nc.scalar.activation(Hs[c][:, sl], Hs[c][:, sl],
                                                     Act.Tanh)
                            emit_head_phase(half, q)

    nc.compile()
    return nc


def _numpy_fallback(x, proj_W, proj_b, Wz, bz, Wx, head_W, head_b):
    z = x.astype(np.float32) @ proj_W.T + proj_b
    for l in range(Wz.shape[0]):
        zx = z @ Wx[l].T + bz[l]
        h = np.zeros_like(z)
        for _ in range(T):
            h = np.tanh(h @ Wz[l].T + zx)
        z = h
    return (z @ head_W.T + head_b).astype(np.float32)


def kernel(x, proj_W, proj_b, Wz, bz, Wx, head_W, head_b):
    x = np.asarray(x, dtype=np.float32)
    proj_W = np.asarray(proj_W, dtype=np.float32)
    proj_b = np.asarray(proj_b, dtype=np.float32)
    Wz = np.asarray(Wz, dtype=np.float32)
    bz = np.asarray(bz, dtype=np.float32)
    Wx = np.asarray(Wx, dtype=np.float32)
    head_W = np.asarray(head_W, dtype=np.float32)
    head_b = np.asarray(head_b, dtype=np.float32)

    # The device kernel folds h @ Wz.T into 0.5*h. Verify that structure holds
    # for these inputs; otherwise compute on host.
    eye = 0.5 * np.eye(D, dtype=np.float32)
    if x.shape != (B, D) or Wz.shape != (L, D, D) or \
            max(np.abs(Wz[l] - eye).max() for l in range(L)) > 1e-6:
        return _numpy_fallback(x, proj_W, proj_b, Wz, bz, Wx, head_W, head_b)

    from concourse.bass_utils import run_bass_kernel_spmd

    if "nc" not in _STATE:
        _STATE["nc"] = _build()
    nc = _STATE["nc"]

    xt = np.ascontiguousarray(x.T)                       # [512, 32768]
    shared = {
        "pwt": np.ascontiguousarray(proj_W.T),
        "pb": proj_b.reshape(D, 1).copy(),
        "wxt": np.ascontiguousarray(Wx.transpose(0, 2, 1)),
        "bz": bz.reshape(L, D, 1).copy(),
        "hwt": np.ascontiguousarray(head_W.T),
        "hb": head_b.reshape(D, 1).copy(),
        "eye": np.concatenate([np.eye(128, dtype=np.float32),
                               0.5 * np.eye(128, dtype=np.float32)], axis=1),
    }
    in_maps = [
        {"xt": np.ascontiguousarray(xt[:, c * BC:(c + 1) * BC]), **shared}
        for c in range(NCORES)
    ]
    res = run_bass_kernel_spmd(nc, in_maps, list(range(NCORES)))
    _STATE["last_result"] = res

    out = np.empty((B, D), dtype=np.float32)
    for c in range(NCORES):
        out[c * BC:(c + 1) * BC, :] = res.results[c]["outt"].T
    return out

